# revision 1
# baseline (speedup 1.0000x reference)
"""Trainium2 Bass kernel for nn_MultiHeadAttention (B=4, S=2048, D=1024, H=16).

Sharding: 8 cores, core c handles batch b=c//2 and query-row half qh=c%2
(1024 query rows), with all 16 heads and the full 2048-key context for
that batch.  No collectives are needed: each core produces a disjoint
[1024, 1024] slab of the output.

Per-core dataflow (all matmuls in fp32r = full PE rate at FP22 precision):
  Phase A: transpose inputs via PE, project Q/K/V.
           - Q.T, K.T produced feature-major ([d, token]); Q.T spilled to a
             DRAM scratch, K.T resident in SBUF.
           - V produced token-major with a ones column appended per head
             (65-wide head stride) and spilled to DRAM scratch.
  Phase B: per head-pair p (K.T rows are 2 heads x 64 dims = 128 partitions):
           scores.T[k, q] via row-packed matmuls (2 heads concurrently on the
           PE via tile_position row groups), exp on the Scalar engine with the
           1/sqrt(64) folded into the activation scale, then x_aug = V_aug.T @ P
           which yields x.T rows 0..63 and the softmax denominator in row 64.
           Normalization: reciprocal of row 64 on the Vector engine, GPSIMD
           partition-broadcast across the 64 head dims, multiply on DVE.
  Phase C: output projection out = x.T.T @ Wo + bo, accumulated over all 8
           head-pair row blocks, written back token-major.
"""

import os
import sys

import numpy as np

sys.path.insert(0, "/opt/trn_rl_repo")

import concourse.bass as bass  # noqa: E402
import concourse.tile as tile  # noqa: E402
from concourse import bacc, mybir  # noqa: E402
from concourse.bass_utils import run_bass_kernel_spmd  # noqa: E402
from concourse.masks import make_identity  # noqa: E402

B, S, D, H = 4, 2048, 1024, 16
HD = D // H          # 64
P = 128
SQ = S // 2          # query rows per core
SK = S               # key rows per core
NIT = D // P         # 8 input-feature tiles
NOT = D // P         # 8 output-feature tiles
KT = SK // P         # 16 key-token tiles
NQB = SQ // 512      # 2 query blocks of 512
NP = H // 2          # 8 head pairs
VW = HD + 1          # 65: head slice of V plus ones column

F32 = mybir.dt.float32
F32R = mybir.dt.float32r
EXP = mybir.ActivationFunctionType.Exp

_CACHE: dict = {}


def _r(ap):
    """fp32r view of an fp32 AP (full-rate PE matmul, FP22 mantissa)."""
    return ap.bitcast(F32R)


def _emit(tc, io):
    nc = tc.nc

    with (
        tc.tile_pool(name="persist", bufs=1) as persist,
        tc.tile_pool(name="consts", bufs=1) as consts,
    ):
        # K.T resident: [dim-in-pair(128), pair, key-token]
        kt_sb = persist.tile([P, NP, SK], F32, tag="ktr")

        # The BIR verifier requires every producer of an fp32r matmul operand
        # to write with an fp32r-typed output, hence the copies below and the
        # _r() on DMA/compute outputs throughout.
        ident_f32 = consts.tile([P, P], F32, tag="identf")
        make_identity(nc, ident_f32)
        ident = consts.tile([P, P], F32, tag="ident")
        nc.vector.tensor_copy(_r(ident[:]), ident_f32[:])
        # biases in per-partition layout: b*[ot*128 + p] = tile[p, ot]
        bqt = consts.tile([P, NOT], F32, tag="bqt")
        nc.sync.dma_start(out=bqt[:], in_=io["bq"].rearrange("(a p) -> p a", p=P))
        bkt = consts.tile([P, NOT], F32, tag="bkt")
        nc.sync.dma_start(out=bkt[:], in_=io["bk"].rearrange("(a p) -> p a", p=P))
        bv_row = consts.tile([1, D], F32, tag="bvr")
        nc.sync.dma_start(out=bv_row[:], in_=io["bv"].rearrange("(a d) -> a d", a=1))
        bo_row = consts.tile([1, D], F32, tag="bor")
        nc.sync.dma_start(out=bo_row[:], in_=io["bo"].rearrange("(a d) -> a d", a=1))
        # biases broadcast to all 128 partitions once (GPSIMD) so projection
        # evictions can add them with a tensor_tensor instead of K=1 matmuls
        bv_bcast = consts.tile([P, D], F32, tag="bvb")
        nc.gpsimd.partition_broadcast(bv_bcast[:], bv_row[0:1, :])
        bo_bcast = consts.tile([P, D], F32, tag="bob")
        nc.gpsimd.partition_broadcast(bo_bcast[:], bo_row[0:1, :])

        # ---------------- Phase A: Q/K/V projections ----------------
        with (
            tc.tile_pool(name="wbuf", bufs=2) as wpool,
            tc.tile_pool(name="xrow", bufs=4) as xrow_pool,
            tc.tile_pool(name="xtblk", bufs=2) as xt_pool,
            tc.tile_pool(name="astage", bufs=3) as stage_pool,
            tc.tile_pool(name="tp_ps", bufs=5, space="PSUM") as tp_psum,
            tc.tile_pool(name="proj_ps", bufs=3, space="PSUM") as proj_psum,
        ):

            def load_w(which):
                w_sb = wpool.tile([P, NIT, D], F32, tag="w", name=f"w_{which}")
                for it in range(NIT):
                    nc.sync.dma_start(
                        out=_r(w_sb[:, it]), in_=_r(io[which][it * P : (it + 1) * P, :])
                    )
                return w_sb

            def transpose_block(x_ap, t0, nt, dst):
                """dst[:, it, ts*128:...] = x_ap[t0:t0+nt*128, :].T via PE."""
                for ts in range(nt):
                    xrow = xrow_pool.tile([P, D], F32, tag="xrow", name=f"xr_{t0}_{ts}")
                    nc.sync.dma_start(
                        out=_r(xrow[:]),
                        in_=_r(x_ap[t0 + ts * P : t0 + (ts + 1) * P, :]),
                    )
                    for it in range(NIT):
                        tp = tp_psum.tile([P, P], F32, tag="tp", name=f"tp_{ts}_{it}")
                        nc.tensor.transpose(
                            _r(tp[:]), _r(xrow[:, it * P : (it + 1) * P]), _r(ident[:])
                        )
                        # drain PSUM on both DVE and the (phase-A idle) ACT
                        # engine so the copy drain stops gating each block
                        if it % 2 == 0:
                            nc.vector.tensor_copy(
                                _r(dst[:, it, ts * P : (ts + 1) * P]), tp[:]
                            )
                        else:
                            nc.scalar.copy(
                                _r(dst[:, it, ts * P : (ts + 1) * P]), tp[:]
                            )

            # --- Q projection -> qt_dram [o, t] (transposed) ---
            # transpose loads emitted before the weight DMA so the PE's first
            # work isn't queued behind a 4MB weight transfer
            xtq = []
            for tb in range(SQ // 512):
                blk = xt_pool.tile([P, NIT, 512], F32, tag="xt", name=f"xtq_{tb}")
                transpose_block(io["xq"], tb * 512, 4, blk)
                xtq.append(blk)
                if tb == 0:
                    w_sb = load_w("wq")
            for tb in range(SQ // 512):
                xt_blk = xtq[tb]
                for ot in range(NOT):
                    ps = proj_psum.tile([P, 512], F32, tag="pj", name=f"qp_{tb}_{ot}")
                    for it in range(NIT):
                        nc.tensor.matmul(
                            ps[:],
                            _r(w_sb[:, it, ot * P : (ot + 1) * P]),
                            _r(xt_blk[:, it]),
                            start=(it == 0),
                            stop=(it == NIT - 1),
                        )
                    st = stage_pool.tile([P, 512], F32, tag="qs", name=f"qs_{tb}_{ot}")
                    nc.scalar.add(st[:], ps[:], bqt[:, ot : ot + 1])
                    nc.sync.dma_start(
                        out=io["qt_dram"][
                            ot * P : (ot + 1) * P, tb * 512 : (tb + 1) * 512
                        ],
                        in_=st[:],
                    )

            # --- K projection -> kt_sb resident (transposed) ---
            w_sb = load_w("wk")
            cur_k = xt_pool.tile([P, NIT, 512], F32, tag="xt", name="xtk_0")
            transpose_block(io["xk"], 0, 4, cur_k)
            for tb in range(SK // 512):
                xt_blk = cur_k
                if tb + 1 < SK // 512:
                    cur_k = xt_pool.tile(
                        [P, NIT, 512], F32, tag="xt", name=f"xtk_{tb + 1}"
                    )
                    transpose_block(io["xk"], (tb + 1) * 512, 4, cur_k)
                for ot in range(NOT):
                    ps = proj_psum.tile([P, 512], F32, tag="pj", name=f"kp_{tb}_{ot}")
                    for it in range(NIT):
                        nc.tensor.matmul(
                            ps[:],
                            _r(w_sb[:, it, ot * P : (ot + 1) * P]),
                            _r(xt_blk[:, it]),
                            start=(it == 0),
                            stop=(it == NIT - 1),
                        )
                    nc.scalar.add(
                        _r(kt_sb[:, ot, tb * 512 : (tb + 1) * 512]),
                        ps[:],
                        bkt[:, ot : ot + 1],
                    )

            # --- V projection -> v_dram token-major, 65-stride per head ---
            w_sb = load_w("wv")
            cur_v = xt_pool.tile([P, NIT, 512], F32, tag="xt", name="xtv_0")
            transpose_block(io["xv"], 0, 4, cur_v)
            for tb in range(SK // 512):
                xt_blk = cur_v
                if tb + 1 < SK // 512:
                    cur_v = xt_pool.tile(
                        [P, NIT, 512], F32, tag="xt", name=f"xtv_{tb + 1}"
                    )
                    transpose_block(io["xv"], (tb + 1) * 512, 4, cur_v)
                for ts in range(4):
                    kt = tb * 4 + ts
                    for ob in range(2):
                        ps = proj_psum.tile(
                            [P, 512], F32, tag="pj", name=f"vp_{kt}_{ob}"
                        )
                        for it in range(NIT):
                            nc.tensor.matmul(
                                ps[:],
                                _r(xt_blk[:, it, ts * P : (ts + 1) * P]),
                                _r(w_sb[:, it, ob * 512 : (ob + 1) * 512]),
                                start=(it == 0),
                                stop=(it == NIT - 1),
                            )
                        vst = stage_pool.tile(
                            [P, 8, VW], F32, tag="vs", name=f"vs_{kt}_{ob}"
                        )
                        nc.vector.tensor_tensor(
                            _r(vst[:, :, 0:HD]),
                            ps[:].rearrange("p (h c) -> p h c", c=HD),
                            bv_bcast[:, ob * 512 : (ob + 1) * 512].rearrange(
                                "p (h c) -> p h c", c=HD
                            ),
                            op=mybir.AluOpType.add,
                        )
                        nc.vector.memset(vst[:, :, HD : HD + 1], 1.0)
                        nc.sync.dma_start(
                            out=_r(
                                io["v_dram"][
                                    kt * P : (kt + 1) * P,
                                    ob * 8 * VW : (ob + 1) * 8 * VW,
                                ]
                            ),
                            in_=_r(vst[:]),
                        )

        # ---------------- Phase B: attention per head pair ----------------
        v_view = io["v_dram"].rearrange("(k p) (g c) -> p k g c", p=P, c=VW)
        with (
            tc.tile_pool(name="bc", bufs=1) as bc_pool,
            tc.tile_pool(name="qtp", bufs=2) as qtp_pool,
            tc.tile_pool(name="vpair", bufs=2) as vp_pool,
            tc.tile_pool(name="pexp", bufs=4) as p_pool,
            tc.tile_pool(name="rcp", bufs=2) as rcp_pool,
            tc.tile_pool(name="rbs", bufs=2) as rbs_pool,
        ):
            # normalized x.T resident: [dim-in-pair(128), pair, query-token]
            xtn_sb = bc_pool.tile([P, NP, SQ], F32, tag="xtn")
            # prefetch Wo during phase B so phase C doesn't stall on it
            wo_sb = bc_pool.tile([P, NP, D], F32, tag="wo")
            for it in range(NP):
                nc.sync.dma_start(
                    out=_r(wo_sb[:, it]), in_=_r(io["wo"][it * P : (it + 1) * P, :])
                )
            with (
                tc.tile_pool(name="s_ps", bufs=3, space="PSUM") as s_psum,
                tc.tile_pool(name="x_ps", bufs=2, space="PSUM") as x_psum,
            ):
                for pr in range(NP):
                    qtp = qtp_pool.tile([P, SQ], F32, tag="qtp", name=f"qtp_{pr}")
                    nc.sync.dma_start(
                        out=_r(qtp[:]), in_=_r(io["qt_dram"][pr * P : (pr + 1) * P, :])
                    )
                    vp = vp_pool.tile([P, KT, 2, VW], F32, tag="vp", name=f"vp_{pr}")
                    nc.sync.dma_start(
                        out=_r(vp[:]), in_=_r(v_view[:, :, 2 * pr : 2 * pr + 2, :])
                    )
                    for qb in range(NQB):
                        xs = [
                            x_psum.tile(
                                [VW, 512], F32, tag="xa", name=f"x_{pr}_{qb}_{h2}"
                            )
                            for h2 in range(2)
                        ]
                        for kt in range(KT):
                            sp = s_psum.tile(
                                [P, 1024], F32, tag="sp", name=f"sp_{pr}_{qb}_{kt}"
                            )
                            for h2 in range(2):
                                nc.tensor.matmul(
                                    sp[:, h2 * 512 : (h2 + 1) * 512],
                                    _r(
                                        kt_sb[
                                            h2 * HD : (h2 + 1) * HD,
                                            pr,
                                            kt * P : (kt + 1) * P,
                                        ]
                                    ),
                                    _r(
                                        qtp[
                                            h2 * HD : (h2 + 1) * HD,
                                            qb * 512 : (qb + 1) * 512,
                                        ]
                                    ),
                                )
                            pe = p_pool.tile(
                                [P, 1024], F32, tag="pe", name=f"pe_{pr}_{qb}_{kt}"
                            )
                            nc.scalar.activation(_r(pe[:]), sp[:], EXP, scale=1.0 / 8.0)
                            for h2 in range(2):
                                nc.tensor.matmul(
                                    xs[h2][:],
                                    _r(vp[:, kt, h2]),
                                    _r(pe[:, h2 * 512 : (h2 + 1) * 512]),
                                    start=(kt == 0),
                                    stop=(kt == KT - 1),
                                )
                        for h2 in range(2):
                            # evict x_aug to SBUF right away so the PSUM bank
                            # frees for the next q-block's AV accumulation
                            xst = rbs_pool.tile(
                                [VW, 512], F32, tag="xst", name=f"xe_{pr}_{qb}_{h2}"
                            )
                            nc.vector.tensor_copy(xst[:], xs[h2][:])
                            rcp = rcp_pool.tile(
                                [1, 512], F32, tag="rcp", name=f"rc_{pr}_{qb}_{h2}"
                            )
                            nc.vector.reciprocal(rcp[:], xst[HD : HD + 1, :])
                            # denominator broadcast across the 64 head dims on
                            # the otherwise-idle GPSIMD engine
                            rb = rbs_pool.tile(
                                [HD, 512], F32, tag="rb", name=f"rb_{pr}_{qb}_{h2}"
                            )
                            nc.gpsimd.partition_broadcast(rb[:], rcp[0:1, :])
                            nc.vector.tensor_tensor(
                                _r(
                                    xtn_sb[
                                        h2 * HD : (h2 + 1) * HD,
                                        pr,
                                        qb * 512 : (qb + 1) * 512,
                                    ]
                                ),
                                xst[0:HD, :],
                                rb[:],
                                op=mybir.AluOpType.mult,
                            )

            # ---------------- Phase C: output projection ----------------
            with (
                tc.tile_pool(name="ostage", bufs=4) as ost_pool,
                tc.tile_pool(name="o_ps", bufs=6, space="PSUM") as o_psum,
            ):
                for qt in range(SQ // P):
                    for ob in range(2):
                        ps = o_psum.tile(
                            [P, 512], F32, tag="op", name=f"op_{qt}_{ob}"
                        )
                        for pr in range(NP):
                            nc.tensor.matmul(
                                ps[:],
                                _r(xtn_sb[:, pr, qt * P : (qt + 1) * P]),
                                _r(wo_sb[:, pr, ob * 512 : (ob + 1) * 512]),
                                start=(pr == 0),
                                stop=(pr == NP - 1),
                            )
                        st = ost_pool.tile(
                            [P, 512], F32, tag="os", name=f"os_{qt}_{ob}"
                        )
                        nc.vector.tensor_tensor(
                            st[:],
                            ps[:],
                            bo_bcast[:, ob * 512 : (ob + 1) * 512],
                            op=mybir.AluOpType.add,
                        )
                        nc.sync.dma_start(
                            out=io["out"][
                                qt * P : (qt + 1) * P, ob * 512 : (ob + 1) * 512
                            ],
                            in_=st[:],
                        )




def build_module():
    if "nc" in _CACHE:
        return _CACHE["nc"]
    nc = bacc.Bacc("TRN2", target_bir_lowering=False, debug=False, num_devices=8)
    io = {}
    io["xq"] = nc.dram_tensor("xq", [SQ, D], F32, kind="ExternalInput").ap()
    io["xk"] = nc.dram_tensor("xk", [SK, D], F32, kind="ExternalInput").ap()
    io["xv"] = nc.dram_tensor("xv", [SK, D], F32, kind="ExternalInput").ap()
    for w in ("wq", "wk", "wv", "wo"):
        io[w] = nc.dram_tensor(w, [D, D], F32, kind="ExternalInput").ap()
    for b in ("bq", "bk", "bv", "bo"):
        io[b] = nc.dram_tensor(b, [D], F32, kind="ExternalInput").ap()
    io["out"] = nc.dram_tensor("out", [SQ, D], F32, kind="ExternalOutput").ap()
    io["qt_dram"] = nc.dram_tensor("qt_scratch", [D, SQ], F32).ap()
    io["v_dram"] = nc.dram_tensor("v_scratch", [SK, H * VW], F32).ap()

    with tile.TileContext(nc) as tc:
        _emit(tc, io)
    nc.compile()
    _CACHE["nc"] = nc
    return nc


LAST_RESULTS = None


def kernel(query, key, value, Wq, bq, Wk, bk, Wv, bv, Wo, bo):
    global LAST_RESULTS
    nc = build_module()
    query = np.ascontiguousarray(np.asarray(query, np.float32))
    key = np.ascontiguousarray(np.asarray(key, np.float32))
    value = np.ascontiguousarray(np.asarray(value, np.float32))
    shared = {
        "wq": np.ascontiguousarray(np.asarray(Wq, np.float32)),
        "wk": np.ascontiguousarray(np.asarray(Wk, np.float32)),
        "wv": np.ascontiguousarray(np.asarray(Wv, np.float32)),
        "wo": np.ascontiguousarray(np.asarray(Wo, np.float32)),
        "bq": np.ascontiguousarray(np.asarray(bq, np.float32)),
        "bk": np.ascontiguousarray(np.asarray(bk, np.float32)),
        "bv": np.ascontiguousarray(np.asarray(bv, np.float32)),
        "bo": np.ascontiguousarray(np.asarray(bo, np.float32)),
    }
    in_maps = []
    for c in range(8):
        b, qh = divmod(c, 2)
        in_maps.append(
            {
                "xq": np.ascontiguousarray(query[b, qh * SQ : (qh + 1) * SQ]),
                "xk": key[b],
                "xv": value[b],
                **shared,
            }
        )
    try:
        res = run_bass_kernel_spmd(nc, in_maps, core_ids=list(range(8)))
    except ModuleNotFoundError:
        # BASS_TRACE was requested but this container lacks the axon NTFF
        # profiling hook module; rerun with tracing disabled.
        os.environ["BASS_NEVER_TRACE"] = "1"
        res = run_bass_kernel_spmd(nc, in_maps, core_ids=list(range(8)))
    LAST_RESULTS = res
    out = np.empty((B, S, D), np.float32)
    for c in range(8):
        b, qh = divmod(c, 2)
        out[b, qh * SQ : (qh + 1) * SQ] = res.results[c]["out"]
    return out



# revision 2
# speedup vs baseline: 1.9762x; 1.9762x over previous
"""Trainium2 Bass kernel for nn_MultiHeadAttention (B=4, S=2048, D=1024, H=16).

Sharding: 8 cores, core c handles batch b=c//2 and query-row half qh=c%2
(1024 query rows), with all 16 heads and the full 2048-key context for
that batch.  No collectives: each core produces a disjoint [1024, 1024]
slab of the output.

v2: everything on the wire is fp16 (inputs converted host-side), halving
per-core transfer volume from ~40MB to ~20MB.  On-chip:
  Phase A: X.T obtained directly via DMA-transpose (xbar) loads - no PE
           transposes, no PSUM drain copies.  Q.T / K.T / V all kept
           resident in SBUF in fp16 (no DRAM scratch round trips).
  Phase B: per head-pair: scores.T = K.T-slab.T @ Q.T via row-group
           concurrent matmul pairs, exp on ACT (scale 1/8 folded in,
           fp16 output), x_aug = V_aug.T @ P accumulated over key tiles
           with a ones column giving the softmax denominator in row 64.
  Phase C: out = x.T.T @ Wo + bo, fp16 output written straight back.
"""

import os
import sys

import numpy as np

sys.path.insert(0, "/opt/trn_rl_repo")

import concourse.bass as bass  # noqa: E402
import concourse.tile as tile  # noqa: E402
from concourse import bacc, mybir  # noqa: E402
from concourse.bass_utils import run_bass_kernel_spmd  # noqa: E402

B, S, D, H = 4, 2048, 1024, 16
HD = D // H          # 64
P = 128
SQ = S // 2          # query rows per core
SK = S               # key rows per core
NIT = D // P         # 8 input-feature tiles
NOT = D // P         # 8 output-feature tiles
KT = SK // P         # 16 key-token tiles
NP = H // 2          # 8 head pairs
VW = HD + 1          # 65: head slice of V plus ones column

F32 = mybir.dt.float32
F16 = mybir.dt.float16
EXP = mybir.ActivationFunctionType.Exp
ADD = mybir.AluOpType.add
MULT = mybir.AluOpType.mult

_CACHE: dict = {}


def _emit(tc, io):
    nc = tc.nc

    with (
        tc.tile_pool(name="persist", bufs=1) as persist,
        tc.tile_pool(name="consts", bufs=1) as consts,
    ):
        # K.T resident: [dim-in-pair(128), pair, key-token], fp16
        kt_sb = persist.tile([P, NP, SK], F16, tag="ktr")
        # V resident token-major with ones column: [tok%128, kt, head, 65]
        v_sb = persist.tile([P, KT, H, VW], F16, tag="vr")
        # Q.T resident: [dim-in-pair, pair, query-token]
        qt_sb = persist.tile([P, NP, SQ], F16, tag="qtr")
        # normalized x.T resident (phase B -> C)
        xtn_sb = persist.tile([P, NP, SQ], F16, tag="xtn")
        # Wo resident (loaded at phase B start)
        wo_sb = persist.tile([P, NP, D], F16, tag="wo")

        # biases in per-partition layout: b*[ot*128 + p] = tile[p, ot]
        bqt = consts.tile([P, NOT], F32, tag="bqt")
        nc.sync.dma_start(out=bqt[:], in_=io["bq"].rearrange("(a p) -> p a", p=P))
        bkt = consts.tile([P, NOT], F32, tag="bkt")
        nc.sync.dma_start(out=bkt[:], in_=io["bk"].rearrange("(a p) -> p a", p=P))
        bv_row = consts.tile([1, D], F32, tag="bvr")
        nc.sync.dma_start(out=bv_row[:], in_=io["bv"].rearrange("(a d) -> a d", a=1))
        bo_row = consts.tile([1, D], F32, tag="bor")
        nc.sync.dma_start(out=bo_row[:], in_=io["bo"].rearrange("(a d) -> a d", a=1))
        bv_bcast = consts.tile([P, D], F32, tag="bvb")
        nc.gpsimd.partition_broadcast(bv_bcast[:], bv_row[0:1, :])
        bo_bcast = consts.tile([P, D], F32, tag="bob")
        nc.gpsimd.partition_broadcast(bo_bcast[:], bo_row[0:1, :])

        # ones column of V_aug, written once
        nc.vector.memset(v_sb[:, :, :, HD : HD + 1], 1.0)

        # ---------------- Phase A: Q/K/V projections ----------------
        with (
            tc.tile_pool(name="wbuf", bufs=2) as wpool,
            tc.tile_pool(name="xtblk", bufs=2) as xt_pool,
            tc.tile_pool(name="proj_ps", bufs=3, space="PSUM") as proj_psum,
        ):

            def load_w(which):
                w_sb = wpool.tile([P, NIT, D], F16, tag="w", name=f"w_{which}")
                for it in range(NIT):
                    nc.sync.dma_start(
                        out=w_sb[:, it], in_=io[which][it * P : (it + 1) * P, :]
                    )
                return w_sb

            def load_xt(x_ap, t0, name):
                """Transpose-load 1024 tokens x D of x_ap into [P, NIT, 1024]."""
                blk = xt_pool.tile([P, NIT, 1024], F16, tag="xt", name=name)
                for it in range(NIT):
                    nc.sync.dma_start(
                        out=blk[:, it],
                        in_=x_ap[t0 : t0 + 1024, it * P : (it + 1) * P],
                        transpose=True,
                    )
                return blk

            # --- K projection -> kt_sb resident (feature-major) ---
            w_sb = load_w("wk")
            cur = load_xt(io["xk"], 0, "xtk_0")
            for tb in range(2):
                xt_blk = cur
                if tb == 0:
                    cur = load_xt(io["xk"], 1024, "xtk_1")
                for ot in range(NOT):
                    ps = proj_psum.tile([P, 1024], F32, tag="pj", name=f"kp_{tb}_{ot}")
                    for it in range(NIT):
                        for hf in range(2):
                            nc.tensor.matmul(
                                ps[:, hf * 512 : (hf + 1) * 512],
                                w_sb[:, it, ot * P : (ot + 1) * P],
                                xt_blk[:, it, hf * 512 : (hf + 1) * 512],
                                start=(it == 0),
                                stop=(it == NIT - 1),
                            )
                    dst = kt_sb[:, ot, tb * 1024 : (tb + 1) * 1024]
                    if ot % 2 == 0:
                        nc.scalar.add(dst, ps[:], bkt[:, ot : ot + 1])
                    else:
                        nc.vector.tensor_scalar_add(dst, ps[:], bkt[:, ot : ot + 1])

            # --- V projection -> v_sb resident (token-major, 65 stride) ---
            w_sb = load_w("wv")
            cur = load_xt(io["xv"], 0, "xtv_0")
            for tb in range(2):
                xt_blk = cur
                if tb == 0:
                    cur = load_xt(io["xv"], 1024, "xtv_1")
                for ts in range(8):
                    kt = tb * 8 + ts
                    ps = proj_psum.tile([P, 1024], F32, tag="pj", name=f"vp_{kt}")
                    for it in range(NIT):
                        for ob in range(2):
                            nc.tensor.matmul(
                                ps[:, ob * 512 : (ob + 1) * 512],
                                xt_blk[:, it, ts * P : (ts + 1) * P],
                                w_sb[:, it, ob * 512 : (ob + 1) * 512],
                                start=(it == 0),
                                stop=(it == NIT - 1),
                            )
                    nc.vector.tensor_tensor(
                        v_sb[:, kt, :, 0:HD],
                        ps[:].rearrange("p (h c) -> p h c", c=HD),
                        bv_bcast[:].rearrange("p (h c) -> p h c", c=HD),
                        op=ADD,
                    )

            # --- Q projection -> qt_sb resident (feature-major) ---
            w_sb = load_w("wq")
            xt_blk = load_xt(io["xq"], 0, "xtq")
            for ot in range(NOT):
                ps = proj_psum.tile([P, 1024], F32, tag="pj", name=f"qp_{ot}")
                for it in range(NIT):
                    for hf in range(2):
                        nc.tensor.matmul(
                            ps[:, hf * 512 : (hf + 1) * 512],
                            w_sb[:, it, ot * P : (ot + 1) * P],
                            xt_blk[:, it, hf * 512 : (hf + 1) * 512],
                            start=(it == 0),
                            stop=(it == NIT - 1),
                        )
                dst = qt_sb[:, ot, :]
                if ot % 2 == 0:
                    nc.scalar.add(dst, ps[:], bqt[:, ot : ot + 1])
                else:
                    nc.vector.tensor_scalar_add(dst, ps[:], bqt[:, ot : ot + 1])

        # ---------------- Phase B: attention per head pair ----------------
        for it in range(NP):
            nc.sync.dma_start(
                out=wo_sb[:, it], in_=io["wo"][it * P : (it + 1) * P, :]
            )
        with (
            tc.tile_pool(name="pexp", bufs=4) as p_pool,
            tc.tile_pool(name="rcp", bufs=2) as rcp_pool,
            tc.tile_pool(name="rbs", bufs=4) as rbs_pool,
            tc.tile_pool(name="s_ps", bufs=2, space="PSUM") as s_psum,
            tc.tile_pool(name="x_ps", bufs=2, space="PSUM") as x_psum,
            tc.tile_pool(name="ostage", bufs=4) as ost_pool,
            tc.tile_pool(name="o_ps", bufs=2, space="PSUM") as o_psum,
        ):
            for pr in range(NP):
                for qb in range(2):
                    xs = [
                        x_psum.tile([VW, 512], F32, tag="xa", name=f"x_{pr}_{qb}_{h2}")
                        for h2 in range(2)
                    ]
                    for kt in range(KT):
                        sp = s_psum.tile(
                            [P, 1024], F32, tag="sp", name=f"sp_{pr}_{qb}_{kt}"
                        )
                        for h2 in range(2):
                            nc.tensor.matmul(
                                sp[:, h2 * 512 : (h2 + 1) * 512],
                                kt_sb[
                                    h2 * HD : (h2 + 1) * HD,
                                    pr,
                                    kt * P : (kt + 1) * P,
                                ],
                                qt_sb[
                                    h2 * HD : (h2 + 1) * HD,
                                    pr,
                                    qb * 512 : (qb + 1) * 512,
                                ],
                            )
                        pe = p_pool.tile(
                            [P, 1024], F16, tag="pe", name=f"pe_{pr}_{qb}_{kt}"
                        )
                        nc.scalar.activation(pe[:], sp[:], EXP, scale=1.0 / 8.0)
                        for h2 in range(2):
                            nc.tensor.matmul(
                                xs[h2][:],
                                v_sb[:, kt, 2 * pr + h2, :],
                                pe[:, h2 * 512 : (h2 + 1) * 512],
                                start=(kt == 0),
                                stop=(kt == KT - 1),
                            )
                    for h2 in range(2):
                        xst = rbs_pool.tile(
                            [VW, 512], F32, tag="xst", name=f"xe_{pr}_{qb}_{h2}"
                        )
                        nc.vector.tensor_copy(xst[:], xs[h2][:])
                        rcp = rcp_pool.tile(
                            [1, 512], F32, tag="rcp", name=f"rc_{pr}_{qb}_{h2}"
                        )
                        nc.vector.reciprocal(rcp[:], xst[HD : HD + 1, :])
                        rb = rbs_pool.tile(
                            [HD, 512], F32, tag="rb", name=f"rb_{pr}_{qb}_{h2}"
                        )
                        nc.gpsimd.partition_broadcast(rb[:], rcp[0:1, :])
                        nc.vector.tensor_tensor(
                            xtn_sb[
                                h2 * HD : (h2 + 1) * HD,
                                pr,
                                qb * 512 : (qb + 1) * 512,
                            ],
                            xst[0:HD, :],
                            rb[:],
                            op=MULT,
                        )

            # ---------------- Phase C: output projection ----------------
            for qt in range(SQ // P):
                for ob in range(2):
                    ps = o_psum.tile([P, 512], F32, tag="op", name=f"op_{qt}_{ob}")
                    for pr in range(NP):
                        nc.tensor.matmul(
                            ps[:],
                            xtn_sb[:, pr, qt * P : (qt + 1) * P],
                            wo_sb[:, pr, ob * 512 : (ob + 1) * 512],
                            start=(pr == 0),
                            stop=(pr == NP - 1),
                        )
                    st = ost_pool.tile([P, 512], F16, tag="os", name=f"os_{qt}_{ob}")
                    nc.vector.tensor_tensor(
                        st[:],
                        ps[:],
                        bo_bcast[:, ob * 512 : (ob + 1) * 512],
                        op=ADD,
                    )
                    nc.sync.dma_start(
                        out=io["out"][
                            qt * P : (qt + 1) * P, ob * 512 : (ob + 1) * 512
                        ],
                        in_=st[:],
                    )


def build_module():
    if "nc" in _CACHE:
        return _CACHE["nc"]
    nc = bacc.Bacc("TRN2", target_bir_lowering=False, debug=False, num_devices=8)
    io = {}
    io["xq"] = nc.dram_tensor("xq", [SQ, D], F16, kind="ExternalInput").ap()
    io["xk"] = nc.dram_tensor("xk", [SK, D], F16, kind="ExternalInput").ap()
    io["xv"] = nc.dram_tensor("xv", [SK, D], F16, kind="ExternalInput").ap()
    for w in ("wq", "wk", "wv", "wo"):
        io[w] = nc.dram_tensor(w, [D, D], F16, kind="ExternalInput").ap()
    for b in ("bq", "bk", "bv", "bo"):
        io[b] = nc.dram_tensor(b, [D], F32, kind="ExternalInput").ap()
    io["out"] = nc.dram_tensor("out", [SQ, D], F16, kind="ExternalOutput").ap()

    with tile.TileContext(nc) as tc:
        _emit(tc, io)
    nc.compile()
    _CACHE["nc"] = nc
    return nc


LAST_RESULTS = None


def kernel(query, key, value, Wq, bq, Wk, bk, Wv, bv, Wo, bo):
    global LAST_RESULTS
    nc = build_module()
    q16 = np.asarray(query, np.float32).astype(np.float16)
    k16 = np.asarray(key, np.float32).astype(np.float16)
    v16 = np.asarray(value, np.float32).astype(np.float16)
    shared = {
        "wq": np.asarray(Wq, np.float32).astype(np.float16),
        "wk": np.asarray(Wk, np.float32).astype(np.float16),
        "wv": np.asarray(Wv, np.float32).astype(np.float16),
        "wo": np.asarray(Wo, np.float32).astype(np.float16),
        "bq": np.ascontiguousarray(np.asarray(bq, np.float32)),
        "bk": np.ascontiguousarray(np.asarray(bk, np.float32)),
        "bv": np.ascontiguousarray(np.asarray(bv, np.float32)),
        "bo": np.ascontiguousarray(np.asarray(bo, np.float32)),
    }
    in_maps = []
    for c in range(8):
        b, qh = divmod(c, 2)
        in_maps.append(
            {
                "xq": np.ascontiguousarray(q16[b, qh * SQ : (qh + 1) * SQ]),
                "xk": np.ascontiguousarray(k16[b]),
                "xv": np.ascontiguousarray(v16[b]),
                **shared,
            }
        )
    try:
        res = run_bass_kernel_spmd(nc, in_maps, core_ids=list(range(8)))
    except ModuleNotFoundError:
        # BASS_TRACE was requested but this container lacks the axon NTFF
        # profiling hook module; rerun with tracing disabled.
        os.environ["BASS_NEVER_TRACE"] = "1"
        res = run_bass_kernel_spmd(nc, in_maps, core_ids=list(range(8)))
    LAST_RESULTS = res
    out = np.empty((B, S, D), np.float32)
    for c in range(8):
        b, qh = divmod(c, 2)
        out[b, qh * SQ : (qh + 1) * SQ] = res.results[c]["out"].astype(np.float32)
    return out
